# revision 15
# baseline (speedup 1.0000x reference)
"""Trainium2 Bass kernel for nn_DynamicAttention (trilinear attention).

Reference computation (per batch b):
    q     = query @ Wq + bq                  # [l, h]
    s_key = src @ Ws + bs                    # [s, h]
    t_key = trg @ Ws + bs                    # [t, h]
    w[l,s,t] = sum_k q[l,k] s_key[s,k] t_key[t,k] / sqrt(h)
    w     = softmax over (s,t)
    s_ctx = sum_{s,t} w * src[s,:] ; t_ctx = sum_{s,t} w * trg[t,:]
    out   = concat(query, s_ctx, t_ctx) @ Wo + bo

Sharding: data-parallel over batch. B=16 across 8 cores -> 2 batches/core.

Device algorithm per core (l=s=t=128, h=512, per batch):
  - Projections as transposed matmuls: qT[h,l], s_keyT[h,s], t_keyT[h,t]
    (inputs arrive host-pre-transposed; weights used as natural lhsT).
  - Scores laid out [l, s*t], computed in 32 chunks of 512 (4 s-values):
    M[k, s*128+t] = s_keyT[k,s]*t_keyT[k,t] built on DVE via broadcast APs,
    then 4 accumulating fp32r matmuls (lhsT=qT chunk, rhs=M chunk).
  - exp on ACT straight out of PSUM (scale=1/sqrt(h)) with accum_out
    giving the per-chunk softmax-denominator partials; E stored bf16.
  - ws[l,s]=sum_t E (per-chunk X-reduce), wt[l,t]=sum_s E (strided reduce),
    scaled by 1/Z, transposed on the PE, then contexts as single matmuls:
    s_ctxT[f,l] = src^T-free matmul(lhsT=src[s,f-chunk], rhs=wsT[s,l]).
  - Final: out[l,:] = sum_j X^T[j-chunk,l]^T @ Wo[j-chunk,:] + bo with
    X^T = [queryT; s_ctxT; t_ctxT] (queryT is the host-provided raw input).
"""

import math

import numpy as np

import concourse.bass as bass
import concourse.bacc as bacc
import concourse.mybir as mybir
import concourse.tile as tile
from concourse.bass_utils import run_bass_kernel_spmd
from concourse.masks import make_identity

F32 = mybir.dt.float32
F32R = mybir.dt.float32r
BF16 = mybir.dt.bfloat16

B, L, S, T = 16, 128, 128, 128
HID, QDIM, FDIM, ODIM = 512, 512, 512, 512
NCORES = 8
BPC = B // NCORES  # batches per core
P = 128
KC = HID // P          # 4 contraction chunks
SBLK = 8               # s-values per score chunk
NCHUNK = S // SBLK     # 32 chunks, each [128, SBLK*128=512]
CW = SBLK * T          # chunk width = 512
JC = (QDIM + 2 * FDIM) // P  # 12 final-matmul contraction chunks

# Per score chunk there are KC=4 M-build planes; the last ACT_PLANES of
# them run on the scalar engine (as SBLK small ops each), the rest on DVE.
ACT_PLANES = 1
# wt-accumulation executor per chunk index: every WT_DVE_EVERYth chunk on
# DVE, the rest on GPSIMD.
WT_DVE_EVERY = 4


def _build_nc():
    nc = bacc.Bacc("TRN2", target_bir_lowering=False, debug=False)

    d_qT = nc.dram_tensor("qT", [QDIM, BPC * L], BF16, kind="ExternalInput")
    d_qTf = nc.dram_tensor("qTf", [QDIM, BPC * L], F32, kind="ExternalInput")
    d_sT = nc.dram_tensor("sT", [FDIM, BPC * S], BF16, kind="ExternalInput")
    d_tT = nc.dram_tensor("tT", [FDIM, BPC * T], BF16, kind="ExternalInput")
    d_src = nc.dram_tensor("src", [BPC * S, FDIM], F32, kind="ExternalInput")
    d_trg = nc.dram_tensor("trg", [BPC * T, FDIM], F32, kind="ExternalInput")
    d_Wq = nc.dram_tensor("Wq", [QDIM, HID], BF16, kind="ExternalInput")
    d_Ws = nc.dram_tensor("Ws", [FDIM, HID], BF16, kind="ExternalInput")
    d_Wo = nc.dram_tensor("Wo", [QDIM + 2 * FDIM, ODIM], F32, kind="ExternalInput")
    d_bq = nc.dram_tensor("bq", [HID], F32, kind="ExternalInput")
    d_bs = nc.dram_tensor("bs", [HID], F32, kind="ExternalInput")
    d_bo = nc.dram_tensor("bo", [ODIM], F32, kind="ExternalInput")
    d_out = nc.dram_tensor("out", [BPC * L, ODIM], F32, kind="ExternalOutput")

    N = BPC * L  # 256: both batches side by side in the free dim

    with tile.TileContext(nc) as tc:
        with (
            tc.tile_pool(name="const", bufs=1) as const,
            tc.tile_pool(name="acts", bufs=1) as acts,
            tc.tile_pool(name="epool", bufs=BPC) as epool,
            tc.tile_pool(name="mpool", bufs=5) as mpool,
            tc.tile_pool(name="small", bufs=2) as small,
            tc.tile_pool(name="ps_score", bufs=3, space="PSUM") as ps_score,
            tc.tile_pool(name="ps_misc", bufs=2, space="PSUM") as ps_misc,
        ):
            # ---- load constants / inputs ----
            qT_sb = const.tile([P, KC, N], BF16)
            sT_sb = const.tile([P, KC, N], BF16)
            tT_sb = const.tile([P, KC, N], BF16)
            qTf_sb = const.tile([P, KC, N], F32)
            nc.sync.dma_start(out=qT_sb, in_=d_qT.rearrange("(c p) n -> p c n", p=P))
            nc.sync.dma_start(out=qTf_sb, in_=d_qTf.rearrange("(c p) n -> p c n", p=P))
            nc.sync.dma_start(out=sT_sb, in_=d_sT.rearrange("(c p) n -> p c n", p=P))
            nc.sync.dma_start(out=tT_sb, in_=d_tT.rearrange("(c p) n -> p c n", p=P))

            Wq_sb = const.tile([P, KC, HID], BF16)
            Ws_sb = const.tile([P, KC, HID], BF16)
            nc.sync.dma_start(out=Wq_sb, in_=d_Wq.rearrange("(c p) h -> p c h", p=P))
            nc.sync.dma_start(out=Ws_sb, in_=d_Ws.rearrange("(c p) h -> p c h", p=P))
            Wo_sb = const.tile([P, JC, ODIM], F32)
            nc.sync.dma_start(out=Wo_sb, in_=d_Wo.rearrange("(c p) o -> p c o", p=P))

            bq_sb = const.tile([P, KC], F32)
            bs_sb = const.tile([P, KC], F32)
            nc.sync.dma_start(out=bq_sb, in_=d_bq.rearrange("(c p) -> p c", p=P))
            nc.sync.dma_start(out=bs_sb, in_=d_bs.rearrange("(c p) -> p c", p=P))
            bo_sb = const.tile([P, ODIM], F32)
            nc.sync.dma_start(
                out=bo_sb, in_=d_bo[:].unsqueeze(0).broadcast_to((P, ODIM))
            )

            src_sb = const.tile([P, BPC, FDIM], F32)
            trg_sb = const.tile([P, BPC, FDIM], F32)
            nc.sync.dma_start(out=src_sb, in_=d_src.rearrange("(b s) f -> s b f", s=P))
            nc.sync.dma_start(out=trg_sb, in_=d_trg.rearrange("(b s) f -> s b f", s=P))

            ident = const.tile([P, P], F32)
            make_identity(nc, ident[:])

            # ---- projections: xT[h, n] = W^T @ inputT, + bias ----
            q_sb = acts.tile([P, KC, N], BF16)
            sk_sb = acts.tile([P, KC, N], F32)
            tk_sb = acts.tile([P, KC, N], BF16)
            for w_sb, x_sb, b_sb, o_sb in (
                (Wq_sb, qT_sb, bq_sb, q_sb),
                (Ws_sb, sT_sb, bs_sb, sk_sb),
                (Ws_sb, tT_sb, bs_sb, tk_sb),
            ):
                for hc in range(KC):
                    pp = ps_misc.tile([P, N], F32, tag="misc")
                    for kc in range(KC):
                        nc.tensor.matmul(
                            pp[:],
                            w_sb[:, kc, hc * P : (hc + 1) * P],
                            x_sb[:, kc, :],
                            start=(kc == 0),
                            stop=(kc == KC - 1),
                        )
                    nc.scalar.activation(
                        out=o_sb[:, hc, :],
                        in_=pp[:],
                        func=mybir.ActivationFunctionType.Identity,
                        bias=b_sb[:, hc : hc + 1],
                        scale=1.0,
                    )

            ctxT_sb = acts.tile([P, 8, N], F32)
            inv_sqrt_h = 1.0 / math.sqrt(HID)

            for b in range(BPC):
                bsl = slice(b * P, (b + 1) * P)
                e_b = epool.tile([P, S, T], BF16, tag="e")
                ws = small.tile([P, S], F32, tag="ws")
                wt512 = small.tile([P, SBLK, T], F32, tag="wt512")
                wt = small.tile([P, T], F32, tag="wt")

                for j in range(NCHUNK):
                    m_t = mpool.tile([P, KC, SBLK, T], BF16, tag="m")
                    scol = slice(b * P + SBLK * j, b * P + SBLK * (j + 1))
                    for kc in range(KC):
                        if kc >= KC - ACT_PLANES:
                            for sj in range(SBLK):
                                col = b * P + SBLK * j + sj
                                nc.scalar.mul(
                                    out=m_t[:, kc, sj, :],
                                    in_=tk_sb[:, kc, bsl],
                                    mul=sk_sb[:, kc, col : col + 1],
                                )
                        else:
                            nc.vector.tensor_tensor(
                                out=m_t[:, kc],
                                in0=tk_sb[:, kc, bsl].unsqueeze(1)
                                .broadcast_to((P, SBLK, T)),
                                in1=sk_sb[:, kc, scol].unsqueeze(2)
                                .broadcast_to((P, SBLK, T)),
                                op=mybir.AluOpType.mult,
                            )
                    sc_ps = ps_score.tile([P, CW], F32, tag="sc")
                    for h in range(CW // 512):
                        for kc in range(KC):
                            nc.tensor.matmul(
                                sc_ps[:, 512 * h : 512 * (h + 1)],
                                q_sb[:, kc, bsl],
                                m_t[:, kc]
                                .rearrange("p s t -> p (s t)")[
                                    :, 512 * h : 512 * (h + 1)
                                ],
                                start=(kc == 0),
                                stop=(kc == KC - 1),
                            )
                    e_chunk = e_b[:, SBLK * j : SBLK * (j + 1), :]
                    nc.scalar.activation(
                        out=e_chunk.rearrange("p s t -> p (s t)"),
                        in_=sc_ps[:],
                        func=mybir.ActivationFunctionType.Exp,
                        scale=inv_sqrt_h,
                    )
                    # wt accumulation: wt512 += sum-over-chunk
                    wt_eng = nc.vector if j % WT_DVE_EVERY == 0 else nc.gpsimd
                    if j == 0:
                        wt_eng.tensor_copy(out=wt512[:], in_=e_chunk)
                    else:
                        wt_eng.tensor_tensor(
                            out=wt512[:], in0=wt512[:], in1=e_chunk,
                            op=mybir.AluOpType.add,
                        )

                # ws[l,s] = sum_t E, in 4 big reduces
                for g in range(4):
                    nc.vector.tensor_reduce(
                        out=ws[:, 32 * g : 32 * (g + 1)],
                        in_=e_b[:, 32 * g : 32 * (g + 1), :],
                        axis=mybir.AxisListType.X,
                        op=mybir.AluOpType.add,
                    )
                # fold wt512 [P,SBLK,T] -> wt [P,T] (binary tree)
                width = SBLK
                while width > 2:
                    half = width // 2
                    nc.gpsimd.tensor_tensor(
                        out=wt512[:, :half],
                        in0=wt512[:, :half],
                        in1=wt512[:, half:width],
                        op=mybir.AluOpType.add,
                    )
                    width = half
                nc.gpsimd.tensor_tensor(
                    out=wt[:], in0=wt512[:, 0], in1=wt512[:, 1],
                    op=mybir.AluOpType.add,
                )

                # softmax denominator and marginals
                z = small.tile([P, 1], F32, tag="z")
                nc.vector.tensor_reduce(
                    out=z[:], in_=ws[:], axis=mybir.AxisListType.X,
                    op=mybir.AluOpType.add,
                )
                invz = small.tile([P, 1], F32, tag="invz")
                nc.vector.reciprocal(out=invz[:], in_=z[:])
                nc.vector.tensor_scalar_mul(ws[:], ws[:], invz[:])
                nc.vector.tensor_scalar_mul(wt[:], wt[:], invz[:])

                # transpose marginals: wsT[s, l], wtT[t, l]
                wsT = small.tile([P, P], F32, tag="wsT")
                wtT = small.tile([P, P], F32, tag="wtT")
                for w_in, w_out in ((ws, wsT), (wt, wtT)):
                    tp = ps_misc.tile([P, P], F32, tag="misc")
                    nc.tensor.transpose(tp[:], w_in[:], ident[:])
                    nc.vector.tensor_copy(out=w_out[:], in_=tp[:])

                # contexts: s_ctxT[f,l] = sum_s src[s,f]*wsT[s,l]
                for w_t, x_sb, off in ((wsT, src_sb, 0), (wtT, trg_sb, 4)):
                    for fc in range(KC):
                        cp = ps_misc.tile([P, P], F32, tag="misc")
                        nc.tensor.matmul(
                            cp[:],
                            x_sb[:, b, fc * P : (fc + 1) * P],
                            w_t[:],
                            start=True,
                            stop=True,
                        )
                        nc.scalar.activation(
                            out=ctxT_sb[:, off + fc, bsl],
                            in_=cp[:],
                            func=mybir.ActivationFunctionType.Identity,
                            scale=1.0,
                        )

                # final: out[l, o] = X^T.T @ Wo + bo
                op_ps = ps_misc.tile([P, ODIM], F32, tag="misc")
                for jc in range(JC):
                    lhsT = (
                        qTf_sb[:, jc, bsl] if jc < KC else ctxT_sb[:, jc - KC, bsl]
                    )
                    nc.tensor.matmul(
                        op_ps[:], lhsT, Wo_sb[:, jc, :],
                        start=(jc == 0), stop=(jc == JC - 1),
                    )
                out_sb = small.tile([P, ODIM], F32, tag="out")
                nc.vector.tensor_tensor(
                    out=out_sb[:], in0=op_ps[:], in1=bo_sb[:],
                    op=mybir.AluOpType.add,
                )
                nc.sync.dma_start(out=d_out[bsl, :], in_=out_sb[:])

    nc.compile()
    return nc


_NC_CACHE = None


def _get_nc():
    global _NC_CACHE
    if _NC_CACHE is None:
        _NC_CACHE = _build_nc()
    return _NC_CACHE




def _core_in_map(tensors, c):
    import ml_dtypes

    bf = ml_dtypes.bfloat16
    sl = slice(BPC * c, BPC * (c + 1))
    qs = np.asarray(tensors["query"], np.float32)[sl].reshape(BPC * L, QDIM)
    ss = np.asarray(tensors["src"], np.float32)[sl].reshape(BPC * S, FDIM)
    ts = np.asarray(tensors["trg"], np.float32)[sl].reshape(BPC * T, FDIM)
    return {
        "qT": np.ascontiguousarray(qs.T).astype(bf),
        "qTf": np.ascontiguousarray(qs.T),
        "sT": np.ascontiguousarray(ss.T).astype(bf),
        "tT": np.ascontiguousarray(ts.T).astype(bf),
        "src": np.ascontiguousarray(ss),
        "trg": np.ascontiguousarray(ts),
        "Wq": np.asarray(tensors["Wq"], np.float32).astype(bf),
        "Ws": np.asarray(tensors["Ws"], np.float32).astype(bf),
        "Wo": np.ascontiguousarray(np.asarray(tensors["Wo"], np.float32)),
        "bq": np.ascontiguousarray(np.asarray(tensors["bq"], np.float32)),
        "bs": np.ascontiguousarray(np.asarray(tensors["bs"], np.float32)),
        "bo": np.ascontiguousarray(np.asarray(tensors["bo"], np.float32)),
    }

def kernel(query, src, trg, Wq, bq, Ws, bs, Wo, bo):
    query = np.asarray(query, dtype=np.float32)
    src = np.asarray(src, dtype=np.float32)
    trg = np.asarray(trg, dtype=np.float32)
    Wq = np.ascontiguousarray(np.asarray(Wq, dtype=np.float32))
    Ws = np.ascontiguousarray(np.asarray(Ws, dtype=np.float32))
    Wo = np.ascontiguousarray(np.asarray(Wo, dtype=np.float32))
    bq = np.ascontiguousarray(np.asarray(bq, dtype=np.float32))
    bs = np.ascontiguousarray(np.asarray(bs, dtype=np.float32))
    bo = np.ascontiguousarray(np.asarray(bo, dtype=np.float32))

    nc = _get_nc()
    in_maps = [_core_in_map(locals(), c) for c in range(NCORES)]
    global _last_in_maps
    _last_in_maps = in_maps
    res = run_bass_kernel_spmd(nc, in_maps, list(range(NCORES))).results
    out = np.concatenate(
        [res[c]["out"].reshape(BPC, L, ODIM) for c in range(NCORES)], axis=0
    )
    return out.astype(np.float32)
